# revision 8
# baseline (speedup 1.0000x reference)
"""CNN-MRF loss (retrieval kNN) on 8 Trainium2 NeuronCores.

Reference: cosine-similarity argmax between all 96x96 content patches and
96x96 style patches (3x3xC=128 patches, d=1152), gather matched style
patches, fold (overlap-add), MSE against content features.

Sharding: content-patch axis N split 8 ways (12 grid rows / core), style
replicated.  Each core:
  1. loads the padded style map (128, 98, 98) and its padded content rows
  2. style patch norms: squared-map box-sum (DVE) + ones-matmul channel
     reduction (PE) + Rsqrt (ACT); partition-broadcast via K=1 matmul
  3. similarity S (128 content x 384 style tiles) = sum of 9 shifted
     matmuls (contraction = channels on partitions), accumulated in PSUM,
     scaled by 1/||s|| during PSUM->SBUF copy
  4. row argmax over the full 9216-wide row via DVE max/max_index
  5. indirect-DMA row gather of matched (un-normalized) style patches
  6. PE transposes to channel-major + DVE fold accumulation into a
     14-row output strip
Host: sums the 8 overlapping strips, divides by fold counts, MSE.
"""
import sys
import numpy as np

for _p in ("/opt/trn_rl_repo",):
    if _p not in sys.path:
        sys.path.insert(0, _p)

import concourse.bass as bass
import concourse.bacc as bacc
import concourse.mybir as mybir
from concourse.bass import IndirectOffsetOnAxis
from concourse.bass_utils import run_bass_kernel_spmd
from concourse.tile import TileContext
from concourse.masks import make_identity

F32 = mybir.dt.float32
U32 = mybir.dt.uint32

C = 128          # channels
H = W = 96       # feature-map spatial dims
PW = 3           # patch size
HP = H + 2       # padded spatial
N = H * W        # content patches total (9216)
M = N            # style patches (9216)
D = C * PW * PW  # patch vector length (1152)
NCORES = 8
RPC = H // NCORES       # content grid rows per core (12)
NSH = RPC * W           # content patches per core (1152)
NT = NSH // 128         # n-tiles of 128 per core (9)
MROWS = 4               # style grid rows per m-tile
MW = MROWS * W          # m-tile width (384)
MT = M // MW            # m-tiles (24)
RCHUNK = 12             # style grid rows per norm chunk
NCH = H // RCHUNK       # norm chunks (8)


def ts(i, size):
    return slice(i * size, (i + 1) * size)


def build_program():
    nc = bacc.Bacc()

    cpad = nc.declare_dram_parameter("cpad", [C, RPC + 2, HP], F32, isOutput=False)
    spad = nc.declare_dram_parameter("spad", [C, HP, HP], F32, isOutput=False)
    sprows = nc.declare_dram_parameter("sprows", [M, D], F32, isOutput=False)
    idx_out = nc.declare_dram_parameter("idx_out", [NT, 128, 1], U32, isOutput=True)
    racc_out = nc.declare_dram_parameter(
        "racc_out", [C, RPC + 2, W], F32, isOutput=True
    )

    with TileContext(nc) as tc:
        with (
            tc.tile_pool(name="const", bufs=1) as constp,
            tc.tile_pool(name="big", bufs=1) as bigp,
            tc.tile_pool(name="work", bufs=2) as workp,
            tc.tile_pool(name="psS", bufs=3, space="PSUM") as psS,
            tc.tile_pool(name="psT", bufs=2, space="PSUM") as psT,
            tc.tile_pool(name="psN", bufs=1, space="PSUM") as psN,
        ):
            # ---- constants / loads ----
            ones_col = constp.tile([C, 1], F32)       # for channel reduction
            nc.gpsimd.memset(ones_col[:], 1.0)
            ones_row = constp.tile([1, 128], F32)     # for partition broadcast
            nc.gpsimd.memset(ones_row[:], 1.0)
            ident = constp.tile([128, 128], F32)
            make_identity(nc, ident[:])

            spad_t = bigp.tile([C, HP, HP], F32)
            nc.sync.dma_start(out=spad_t[:], in_=spad[:])
            cpad_t = bigp.tile([C, RPC + 2, HP], F32)
            nc.sync.dma_start(out=cpad_t[:], in_=cpad[:])

            # ---- phase A: style patch inverse norms -> invb (128, M) ----
            invb = bigp.tile([C, M], F32)
            for rc in range(NCH):
                r0 = rc * RCHUNK
                sq = workp.tile([C, RCHUNK + 2, HP], F32, tag="sq")
                nc.vector.tensor_mul(
                    sq[:],
                    spad_t[:, r0 : r0 + RCHUNK + 2, :],
                    spad_t[:, r0 : r0 + RCHUNK + 2, :],
                )
                sqb = workp.tile([C, RCHUNK, W], F32, tag="sqb")
                nc.vector.tensor_add(
                    sqb[:], sq[:, 0:RCHUNK, 0:W], sq[:, 0:RCHUNK, 1 : 1 + W]
                )
                for k in range(2, 9):
                    ki, kj = k // 3, k % 3
                    nc.vector.tensor_add(
                        sqb[:], sqb[:], sq[:, ki : ki + RCHUNK, kj : kj + W]
                    )
                # channel reduction + rsqrt + partition broadcast,
                # m-chunks of MW (=384) columns
                for q in range(RCHUNK // MROWS):
                    psum_n = psN.tile([1, MW], F32, tag="psn")
                    nc.tensor.matmul(
                        out=psum_n[:],
                        lhsT=ones_col[:],
                        rhs=sqb[:, q * MROWS : (q + 1) * MROWS, :],
                        start=True,
                        stop=True,
                    )
                    rec = workp.tile([1, MW], F32, tag="rec")
                    nc.vector.reciprocal(rec[:], psum_n[:])
                    invn = workp.tile([1, MW], F32, tag="invn")
                    nc.scalar.activation(
                        invn[:], rec[:], mybir.ActivationFunctionType.Sqrt
                    )
                    t = rc * (RCHUNK // MROWS) + q
                    psum_b = psN.tile([128, MW], F32, tag="psb")
                    nc.tensor.matmul(
                        out=psum_b[:],
                        lhsT=ones_row[:],
                        rhs=invn[:],
                        start=True,
                        stop=True,
                    )
                    nc.vector.tensor_copy(invb[:, ts(t, MW)], psum_b[:])

            # ---- phase B0: contiguous shifted content views ----
            cshift = bigp.tile([C, 9, NSH], F32)
            for k in range(9):
                ki, kj = k // 3, k % 3
                nc.vector.tensor_copy(
                    cshift[:, k], cpad_t[:, ki : ki + RPC, kj : kj + W]
                )

            # ---- phase B/C: similarity + argmax + gather + fold ----
            racc = bigp.tile([C, RPC + 2, HP], F32)
            nc.gpsimd.memset(racc[:], 0.0)

            S_sb = bigp.tile([C, M], F32)
            for j in range(NT):
                for t in range(MT):
                    mrow = t * MROWS
                    psum_S = psS.tile([128, MW], F32, tag="psS")
                    for k in range(9):
                        ki, kj = k // 3, k % 3
                        nc.tensor.matmul(
                            out=psum_S[:],
                            lhsT=cshift[:, k, ts(j, 128)],
                            rhs=spad_t[:, mrow + ki : mrow + ki + MROWS, kj : kj + W],
                            start=(k == 0),
                            stop=(k == 8),
                        )
                    nc.vector.tensor_mul(
                        S_sb[:, ts(t, MW)], psum_S[:], invb[:, ts(t, MW)]
                    )
                max8 = workp.tile([128, 8], F32, tag="max8")
                nc.vector.max(max8[:], S_sb[:])
                idx8 = workp.tile([128, 8], U32, tag="idx8")
                nc.vector.max_index(idx8[:], max8[:], S_sb[:])
                nc.sync.dma_start(out=idx_out[j], in_=idx8[:, 0:1])

                # gather matched style patch rows (n-major); the indirect
                # DMA needs a flat 2D dest (3D dest tiles fetch garbage)
                matched = workp.tile([128, D], F32, tag="matched")
                nc.gpsimd.indirect_dma_start(
                    out=matched[:],
                    out_offset=None,
                    in_=sprows[:],
                    in_offset=IndirectOffsetOnAxis(ap=idx8[:, 0:1], axis=0),
                )
                matched3 = matched[:].rearrange("p (a b) -> p a b", b=9)

                # transpose to channel-major and fold-accumulate
                n0 = j * 128
                r0, c0 = n0 // W, n0 % W
                seg1 = (r0, c0, W - c0, 0)
                seg2 = (r0 + 1, 0, 128 - (W - c0), W - c0)
                for k in range(9):
                    ki, kj = k // 3, k % 3
                    psum_T = psT.tile([128, 128], F32, tag="psT")
                    nc.tensor.transpose(psum_T[:], matched3[:, :, k], ident[:])
                    for (r, c, ln, off) in (seg1, seg2):
                        nc.vector.tensor_add(
                            racc[:, r + ki, c + kj : c + kj + ln],
                            racc[:, r + ki, c + kj : c + kj + ln],
                            psum_T[:, off : off + ln],
                        )

            nc.sync.dma_start(out=racc_out[:], in_=racc[:, :, 1 : 1 + W])

    if not nc.is_finalized():
        nc.finalize()
    return nc


_PROGRAM = None


def _get_program():
    global _PROGRAM
    if _PROGRAM is None:
        _PROGRAM = build_program()
    return _PROGRAM


def _host_prep(content_feats, style_feats):
    """Build per-core input maps."""
    cf = np.ascontiguousarray(np.asarray(content_feats, dtype=np.float32)[0])
    sf = np.ascontiguousarray(np.asarray(style_feats, dtype=np.float32)[0])
    cpad = np.pad(cf, ((0, 0), (1, 1), (1, 1)))
    spad = np.pad(sf, ((0, 0), (1, 1), (1, 1)))
    # style patch rows: (M, D) with row m=(y,x) = spad[c, y+ki, x+kj],
    # d-index = c*9 + ki*3 + kj  (unfold (c, kh, kw) ordering)
    win = np.lib.stride_tricks.sliding_window_view(spad, (PW, PW), axis=(1, 2))
    sprows = np.ascontiguousarray(
        win.transpose(1, 2, 0, 3, 4).reshape(M, D)
    )
    in_maps = []
    for i in range(NCORES):
        in_maps.append(
            {
                "cpad": np.ascontiguousarray(
                    cpad[:, i * RPC : i * RPC + RPC + 2, :]
                ),
                "spad": spad,
                "sprows": sprows,
            }
        )
    return cf, in_maps


_DIVISOR = None


def _fold_divisor():
    global _DIVISOR
    if _DIVISOR is None:
        cnt = np.full(H, 3, dtype=np.float32)
        cnt[0] = cnt[-1] = 2
        _DIVISOR = np.outer(cnt, cnt).astype(np.float32) + np.float32(1e-8)
    return _DIVISOR


def _host_combine(cf, results):
    acc = np.zeros((C, H + 2, W), dtype=np.float32)
    for i in range(NCORES):
        acc[:, i * RPC : i * RPC + RPC + 2, :] += results[i]["racc_out"]
    recon = acc[:, 1 : 1 + H, :] / _fold_divisor()[None, :, :]
    diff = cf - recon
    return np.float32(np.mean(np.square(diff), dtype=np.float64))


def run(content_feats, style_feats, trace=False):
    nc = _get_program()
    cf, in_maps = _host_prep(content_feats, style_feats)
    res = run_bass_kernel_spmd(
        nc, in_maps, core_ids=list(range(NCORES)), trace=trace
    )
    mse = _host_combine(cf, res.results)
    return mse, res


def kernel(content_feats, style_feats):
    mse, _ = run(content_feats, style_feats)
    return np.array(mse, dtype=np.float32)


# revision 11
# speedup vs baseline: 1.9762x; 1.9762x over previous
"""CNN-MRF loss (retrieval kNN) on 8 Trainium2 NeuronCores.

Reference: cosine-similarity argmax between all 96x96 content patches and
96x96 style patches (3x3xC=128 patches, d=1152), gather matched style
patches, fold (overlap-add), MSE against content features.

Sharding: content-patch axis N split 8 ways (12 grid rows / core), style
replicated.  Each core:
  1. loads the padded style map (128, 98, 98) and its padded content rows
  2. style patch norms: squared-map box-sum (DVE) + ones-matmul channel
     reduction (PE) + Rsqrt (ACT); partition-broadcast via K=1 matmul
  3. similarity S (128 content x 384 style tiles) = sum of 9 shifted
     matmuls (contraction = channels on partitions), accumulated in PSUM,
     scaled by 1/||s|| during PSUM->SBUF copy
  4. row argmax over the full 9216-wide row via DVE max/max_index
  5. indirect-DMA row gather of matched (un-normalized) style patches
  6. PE transposes to channel-major + DVE fold accumulation into a
     14-row output strip
Host: sums the 8 overlapping strips, divides by fold counts, MSE.
"""
import sys
import numpy as np

for _p in ("/opt/trn_rl_repo",):
    if _p not in sys.path:
        sys.path.insert(0, _p)

import concourse.bass as bass
import concourse.bacc as bacc
import concourse.mybir as mybir
from concourse.bass import IndirectOffsetOnAxis
from concourse.bass_utils import run_bass_kernel_spmd
from concourse.tile import TileContext
from concourse.masks import make_identity

F32 = mybir.dt.float32
F32R = mybir.dt.float32r  # relaxed fp32 matmul mode: 4x PE throughput
U32 = mybir.dt.uint32

C = 128          # channels
H = W = 96       # feature-map spatial dims
PW = 3           # patch size
HP = H + 2       # padded spatial
N = H * W        # content patches total (9216)
M = N            # style patches (9216)
D = C * PW * PW  # patch vector length (1152)
NCORES = 8
RPC = H // NCORES       # content grid rows per core (12)
NSH = RPC * W           # content patches per core (1152)
NT = NSH // 128         # n-tiles of 128 per core (9)
MROWS = 4               # style grid rows per m-tile
MW = MROWS * W          # m-tile width (384)
MT = M // MW            # m-tiles (24)
RCHUNK = 12             # style grid rows per norm chunk
NCH = H // RCHUNK       # norm chunks (8)


def ts(i, size):
    return slice(i * size, (i + 1) * size)


def build_program():
    nc = bacc.Bacc()

    cpad = nc.declare_dram_parameter("cpad", [C, RPC + 2, HP], F32R, isOutput=False)
    spad = nc.declare_dram_parameter("spad", [C, HP, HP], F32R, isOutput=False)
    sprows = nc.declare_dram_parameter("sprows", [M, D], F32, isOutput=False)
    idx_out = nc.declare_dram_parameter("idx_out", [NT, 128, 1], U32, isOutput=True)
    racc_out = nc.declare_dram_parameter(
        "racc_out", [C, RPC + 2, W], F32, isOutput=True
    )

    with TileContext(nc) as tc:
        with (
            tc.tile_pool(name="const", bufs=1) as constp,
            tc.tile_pool(name="big", bufs=1) as bigp,
            tc.tile_pool(name="work", bufs=2) as workp,
            tc.tile_pool(name="psS", bufs=3, space="PSUM") as psS,
            tc.tile_pool(name="psT", bufs=2, space="PSUM") as psT,
            tc.tile_pool(name="psN", bufs=1, space="PSUM") as psN,
        ):
            # ---- constants / loads ----
            ones_col = constp.tile([C, 1], F32)       # for channel reduction
            nc.gpsimd.memset(ones_col[:], 1.0)
            ones_row = constp.tile([1, 128], F32)     # for partition broadcast
            nc.gpsimd.memset(ones_row[:], 1.0)
            ident = constp.tile([128, 128], F32)
            make_identity(nc, ident[:])

            spad_t = bigp.tile([C, HP, HP], F32R)
            nc.sync.dma_start(out=spad_t[:], in_=spad[:])
            cpad_t = bigp.tile([C, RPC + 2, HP], F32R)
            nc.sync.dma_start(out=cpad_t[:], in_=cpad[:])

            # ---- phase A: style patch inverse norms -> invb (128, M) ----
            invb = bigp.tile([C, M], F32)
            for rc in range(NCH):
                r0 = rc * RCHUNK
                sq = workp.tile([C, RCHUNK + 2, HP], F32, tag="sq")
                nc.vector.tensor_mul(
                    sq[:],
                    spad_t[:, r0 : r0 + RCHUNK + 2, :].bitcast(F32),
                    spad_t[:, r0 : r0 + RCHUNK + 2, :].bitcast(F32),
                )
                sqb = workp.tile([C, RCHUNK, W], F32, tag="sqb")
                nc.vector.tensor_add(
                    sqb[:], sq[:, 0:RCHUNK, 0:W], sq[:, 0:RCHUNK, 1 : 1 + W]
                )
                for k in range(2, 9):
                    ki, kj = k // 3, k % 3
                    nc.vector.tensor_add(
                        sqb[:], sqb[:], sq[:, ki : ki + RCHUNK, kj : kj + W]
                    )
                # channel reduction + rsqrt + partition broadcast,
                # m-chunks of MW (=384) columns
                for q in range(RCHUNK // MROWS):
                    psum_n = psN.tile([1, MW], F32, tag="psn")
                    nc.tensor.matmul(
                        out=psum_n[:],
                        lhsT=ones_col[:],
                        rhs=sqb[:, q * MROWS : (q + 1) * MROWS, :],
                        start=True,
                        stop=True,
                    )
                    rec = workp.tile([1, MW], F32, tag="rec")
                    nc.vector.reciprocal(rec[:], psum_n[:])
                    invn = workp.tile([1, MW], F32, tag="invn")
                    nc.scalar.activation(
                        invn[:], rec[:], mybir.ActivationFunctionType.Sqrt
                    )
                    t = rc * (RCHUNK // MROWS) + q
                    psum_b = psN.tile([128, MW], F32, tag="psb")
                    nc.tensor.matmul(
                        out=psum_b[:],
                        lhsT=ones_row[:],
                        rhs=invn[:],
                        start=True,
                        stop=True,
                    )
                    nc.vector.tensor_copy(invb[:, ts(t, MW)], psum_b[:])

            # ---- phase B0: contiguous shifted content views ----
            cshift = bigp.tile([C, 9, NSH], F32R)
            for k in range(9):
                ki, kj = k // 3, k % 3
                nc.vector.tensor_copy(
                    cshift[:, k], cpad_t[:, ki : ki + RPC, kj : kj + W]
                )

            # ---- phase B/C: similarity + argmax + gather + fold ----
            racc = bigp.tile([C, RPC + 2, HP], F32)
            nc.gpsimd.memset(racc[:], 0.0)

            S_sb = bigp.tile([C, M], F32)
            for j in range(NT):
                for t in range(MT):
                    mrow = t * MROWS
                    psum_S = psS.tile([128, MW], F32, tag="psS")
                    for k in range(9):
                        ki, kj = k // 3, k % 3
                        nc.tensor.matmul(
                            out=psum_S[:],
                            lhsT=cshift[:, k, ts(j, 128)],
                            rhs=spad_t[
                                :, mrow + ki : mrow + ki + MROWS, kj : kj + W
                            ],
                            start=(k == 0),
                            stop=(k == 8),
                        )
                    nc.vector.tensor_mul(
                        S_sb[:, ts(t, MW)], psum_S[:], invb[:, ts(t, MW)]
                    )
                max8 = workp.tile([128, 8], F32, tag="max8")
                nc.vector.max(max8[:], S_sb[:])
                idx8 = workp.tile([128, 8], U32, tag="idx8")
                nc.vector.max_index(idx8[:], max8[:], S_sb[:])
                nc.sync.dma_start(out=idx_out[j], in_=idx8[:, 0:1])

                # gather matched style patch rows (n-major); the indirect
                # DMA needs a flat 2D dest (3D dest tiles fetch garbage)
                matched = workp.tile([128, D], F32, tag="matched")
                nc.gpsimd.indirect_dma_start(
                    out=matched[:],
                    out_offset=None,
                    in_=sprows[:],
                    in_offset=IndirectOffsetOnAxis(ap=idx8[:, 0:1], axis=0),
                )
                matched3 = matched[:].rearrange("p (a b) -> p a b", b=9)

                # transpose to channel-major and fold-accumulate
                n0 = j * 128
                r0, c0 = n0 // W, n0 % W
                seg1 = (r0, c0, W - c0, 0)
                seg2 = (r0 + 1, 0, 128 - (W - c0), W - c0)
                for k in range(9):
                    ki, kj = k // 3, k % 3
                    psum_T = psT.tile([128, 128], F32, tag="psT")
                    nc.tensor.transpose(psum_T[:], matched3[:, :, k], ident[:])
                    for (r, c, ln, off) in (seg1, seg2):
                        nc.vector.tensor_add(
                            racc[:, r + ki, c + kj : c + kj + ln],
                            racc[:, r + ki, c + kj : c + kj + ln],
                            psum_T[:, off : off + ln],
                        )

            nc.sync.dma_start(out=racc_out[:], in_=racc[:, :, 1 : 1 + W])

    if not nc.is_finalized():
        nc.finalize()
    return nc


_PROGRAM = None


def _get_program():
    global _PROGRAM
    if _PROGRAM is None:
        _PROGRAM = build_program()
    return _PROGRAM


def _host_prep(content_feats, style_feats):
    """Build per-core input maps."""
    cf = np.ascontiguousarray(np.asarray(content_feats, dtype=np.float32)[0])
    sf = np.ascontiguousarray(np.asarray(style_feats, dtype=np.float32)[0])
    cpad = np.pad(cf, ((0, 0), (1, 1), (1, 1)))
    spad = np.pad(sf, ((0, 0), (1, 1), (1, 1)))
    # style patch rows: (M, D) with row m=(y,x) = spad[c, y+ki, x+kj],
    # d-index = c*9 + ki*3 + kj  (unfold (c, kh, kw) ordering)
    win = np.lib.stride_tricks.sliding_window_view(spad, (PW, PW), axis=(1, 2))
    sprows = np.ascontiguousarray(
        win.transpose(1, 2, 0, 3, 4).reshape(M, D)
    )
    in_maps = []
    for i in range(NCORES):
        in_maps.append(
            {
                "cpad": np.ascontiguousarray(
                    cpad[:, i * RPC : i * RPC + RPC + 2, :]
                ),
                "spad": spad,
                "sprows": sprows,
            }
        )
    return cf, in_maps


_DIVISOR = None


def _fold_divisor():
    global _DIVISOR
    if _DIVISOR is None:
        cnt = np.full(H, 3, dtype=np.float32)
        cnt[0] = cnt[-1] = 2
        _DIVISOR = np.outer(cnt, cnt).astype(np.float32) + np.float32(1e-8)
    return _DIVISOR


def _host_combine(cf, results):
    acc = np.zeros((C, H + 2, W), dtype=np.float32)
    for i in range(NCORES):
        acc[:, i * RPC : i * RPC + RPC + 2, :] += results[i]["racc_out"]
    recon = acc[:, 1 : 1 + H, :] / _fold_divisor()[None, :, :]
    diff = cf - recon
    return np.float32(np.mean(np.square(diff), dtype=np.float64))


def run(content_feats, style_feats, trace=False):
    nc = _get_program()
    cf, in_maps = _host_prep(content_feats, style_feats)
    res = run_bass_kernel_spmd(
        nc, in_maps, core_ids=list(range(NCORES)), trace=trace
    )
    mse = _host_combine(cf, res.results)
    return mse, res


def kernel(content_feats, style_feats):
    mse, _ = run(content_feats, style_feats)
    return np.array(mse, dtype=np.float32)
